# revision 23
# baseline (speedup 1.0000x reference)
"""AttentionBlock (GroupNorm + 8-head attention + proj + residual) on 8 TRN2 cores.

Sharding: data-parallel over batch B=8 -> one image per NeuronCore, weights
replicated, no collectives.

Fast path:
 - S = (a*q)k^T via fp8e4m3 DoubleRow matmuls (head_dim 64 = 32 partitions x 2)
 - exp via bit-trick: P_bits = int8(max(S + 32, 0)) viewed as fp8e4m3 == 2^S
   (the 8*log2(e) logit scale is folded into the Q weights on the host)
 - H = V @ P via fp8 DoubleRow (s = 128 partitions x 2), rowsum via ones row
 - QKV / V^T / proj matmuls in bf16, GroupNorm via bn_stats + group-mask matmul
 - bf16 output + on-device residual (x + proj_b), host upcasts
"""
import sys
import types

import numpy as np
import ml_dtypes

import concourse.bass as bass
import concourse.tile as tile
from concourse import bacc, mybir
from concourse.bass_utils import run_bass_kernel_spmd

F32 = mybir.dt.float32
BF16 = mybir.dt.bfloat16
FP8 = mybir.dt.float8e4
I8 = mybir.dt.int8

B, C, N = 8, 512, 1024          # batch, channels, H*W
NH, HD = 8, 64                  # heads, head_dim
G, GS = 32, 16                  # groups, channels per group
EPS = 1e-5
NCORES = 8
CT = C // 128                   # 4 channel tiles
ST = N // 128                   # 8 s-tiles
NCH = 2                         # t-chunks of 512
TRACE = False
DEBUG = False

LOG2E = float(np.log2(np.e))
A8 = 8.0 * LOG2E                # folded into q weights: S_psum = 8*log2e*logits
B8 = 32.0                       # exp-bias for fp8e4m3 bit pattern (2^-3 common factor)

_CACHE = {}


def _install_ntff_hook():
    if "antenv.axon_hooks" in sys.modules:
        return
    try:
        from trn_agent_boot.trn_boot import _ntff_profile_via_ctypes
        hook = _ntff_profile_via_ctypes("/opt/axon/libaxon_pjrt.so")
    except Exception:
        hook = None
    mod = types.ModuleType("antenv.axon_hooks")
    mod.get_axon_ntff_profile_hook = lambda: hook
    mod.set_axon_ntff_profile_hook = lambda h: None
    sys.modules["antenv.axon_hooks"] = mod


def build_nc(debug=False):
    nc = bacc.Bacc("TRN2", target_bir_lowering=False, debug=False,
                   num_devices=NCORES)
    x = nc.dram_tensor("x", (C, N), BF16, kind="ExternalInput").ap()
    qkvw = nc.dram_tensor("qkvw", (C, 12 * 128), BF16, kind="ExternalInput").ap()
    pw = nc.dram_tensor("pw", (C, C), BF16, kind="ExternalInput").ap()
    gnw = nc.dram_tensor("gnw", (128, CT), F32, kind="ExternalInput").ap()
    gnb = nc.dram_tensor("gnb", (128, CT), F32, kind="ExternalInput").ap()
    pb = nc.dram_tensor("pb", (128, CT), F32, kind="ExternalInput").ap()
    mask = nc.dram_tensor("mask", (128, 128), F32, kind="ExternalInput").ap()
    out = nc.dram_tensor("out", (C, N), BF16, kind="ExternalOutput").ap()

    dbg = {}
    if debug:
        for name, shape in [("d_xn", (C, N)), ("d_q", (128, 4 * N)),
                            ("d_k", (128, 4 * N)), ("d_vt", (128, ST * NH * 80)),
                            ("d_h", (C, N)), ("d_p", (128, 8 * N)),
                            ("d_hraw", (66, N)), ("d_rs", (1, N)),
                            ("d_rsb", (64, N)), ("d_ps64", (1, N))]:
            dbg[name] = nc.dram_tensor(name, shape, F32, kind="ExternalOutput").ap()

    x_t = x.rearrange("(t p) n -> p t n", p=128)
    qkvw_t = qkvw.rearrange("(t p) o -> p t o", p=128)
    pw_t = pw.rearrange("(t p) o -> p t o", p=128)
    out_t = out.rearrange("(t p) n -> p t n", p=128)

    with tile.TileContext(nc) as tc:
        with (
            tc.tile_pool(name="wpool", bufs=1) as wp,       # persistent
            tc.tile_pool(name="small", bufs=1) as sm,       # consts/stats
            tc.tile_pool(name="ppool", bufs=14) as pp,      # P fp8 tiles [128,2,1024]
            tc.tile_pool(name="hrawp", bufs=8) as hrawp,    # h_raw bf16 [65,1024]
            tc.tile_pool(name="rsp", bufs=6) as rsp,        # rowsum recip [1,1024]
            tc.tile_pool(name="rsbp", bufs=3) as rsbp,      # broadcast [64,1024]
            tc.tile_pool(name="p1p", bufs=4) as p1p,        # proj partial [128,1024]
            tc.tile_pool(name="outp", bufs=4) as op_,       # output tiles
            tc.tile_pool(name="dbgp", bufs=2) as dbgp,      # debug dumps
            tc.tile_pool(name="ps_mm", bufs=2, space="PSUM") as ps_mm,
            tc.tile_pool(name="ps_s", bufs=2, space="PSUM") as ps_s,
            tc.tile_pool(name="ps_h", bufs=2, space="PSUM") as ps_h,
        ):
            # ---- persistent SBUF ----
            qkvw_sb = wp.tile([128, CT, 12, 128], BF16, tag="qkvw")
            pw_sb = wp.tile([128, CT, C], BF16, tag="pw")
            x_sb = wp.tile([128, CT, N], BF16, tag="xbf")
            xn_sb = wp.tile([128, CT, N], BF16, tag="xn")
            r_sb = wp.tile([128, CT, N], BF16, tag="res")
            q_sb = wp.tile([128, 2, 2, N], FP8, tag="q")   # [p, hg, sub, t]
            k_sb = wp.tile([128, 2, 2, N], FP8, tag="k")
            vt_sb = wp.tile([128, ST, NH, 80], FP8, tag="vt")
            h_sb = wp.tile([128, CT, N], BF16, tag="h")
            gnw_sb = wp.tile([128, CT], F32, tag="gnw")
            gnb_sb = wp.tile([128, CT], F32, tag="gnb")
            pb_sb = wp.tile([128, CT], F32, tag="pb")
            mask_sb = wp.tile([128, 128], F32, tag="mask")

            # ---- input DMAs: spread issue across engines ----
            nc.sync.dma_start(out=qkvw_sb, in_=qkvw_t.rearrange(
                "p t (b o) -> p t b o", b=12))
            nc.scalar.dma_start(out=x_sb[:, 0, :], in_=x_t[:, 0, :])
            nc.scalar.dma_start(out=x_sb[:, 1, :], in_=x_t[:, 1, :])
            nc.gpsimd.dma_start(out=x_sb[:, 2, :], in_=x_t[:, 2, :])
            nc.gpsimd.dma_start(out=x_sb[:, 3, :], in_=x_t[:, 3, :])
            nc.scalar.dma_start(out=mask_sb, in_=mask)
            nc.scalar.dma_start(out=gnw_sb, in_=gnw)
            nc.scalar.dma_start(out=gnb_sb, in_=gnb)
            nc.scalar.dma_start(out=pb_sb, in_=pb)
            nc.sync.dma_start(out=pw_sb, in_=pw_t)
            nc.vector.memset(vt_sb[:, :, :, 64:65], 1.0)
            nc.vector.memset(vt_sb[:, :, :, 65:80], 0.0)

            eps_t = sm.tile([128, 1], F32, tag="eps")
            nc.vector.memset(eps_t, EPS)
            b8_t = sm.tile([128, 1], F32, tag="b8")
            nc.vector.memset(b8_t, B8)

            # ---- GroupNorm stats (vector) ----
            stats_in = sm.tile([128, 8], F32, tag="sin")
            for ct in range(CT):
                stats = sm.tile([128, 2, 6], F32, tag="bst")
                for j in range(2):
                    nc.vector.bn_stats(out=stats[:, j, :],
                                       in_=x_sb[:, ct, j * 512:(j + 1) * 512])
                mv = sm.tile([128, 2], F32, tag="mv")
                nc.vector.bn_aggr(out=mv, in_=stats)
                nc.vector.tensor_copy(stats_in[:, ct:ct + 1], mv[:, 0:1])
                msq = sm.tile([128, 1], F32, tag="msq")
                nc.vector.tensor_mul(msq, mv[:, 0:1], mv[:, 0:1])
                nc.vector.tensor_add(stats_in[:, 4 + ct:5 + ct], mv[:, 1:2], msq)
            stats_ps = ps_mm.tile([128, 8], F32, tag="mm")
            nc.tensor.matmul(stats_ps, mask_sb, stats_in, start=True, stop=True)
            stats_gs = sm.tile([128, 8], F32, tag="sgs")
            nc.vector.tensor_copy(stats_gs, stats_ps)
            means_g = stats_gs[:, 0:4]
            e2_g = stats_gs[:, 4:8]
            msq_g = sm.tile([128, 4], F32, tag="msqg")
            nc.vector.tensor_mul(msq_g, means_g, means_g)
            var_g = sm.tile([128, 4], F32, tag="varg")
            nc.vector.tensor_tensor(out=var_g, in0=e2_g, in1=msq_g,
                                    op=mybir.AluOpType.subtract)
            lnv = sm.tile([128, 4], F32, tag="lnv")
            nc.scalar.activation(out=lnv, in_=var_g,
                                 func=mybir.ActivationFunctionType.Ln,
                                 bias=eps_t, scale=1.0)
            rstd = sm.tile([128, 4], F32, tag="rstd")
            nc.scalar.activation(out=rstd, in_=lnv,
                                 func=mybir.ActivationFunctionType.Exp,
                                 bias=0.0, scale=-0.5)
            sc_g = sm.tile([128, 4], F32, tag="scg")
            nc.vector.tensor_mul(sc_g, rstd, gnw_sb)
            tmp_b = sm.tile([128, 4], F32, tag="tmpb")
            nc.vector.tensor_mul(tmp_b, means_g, sc_g)
            bias_g = sm.tile([128, 4], F32, tag="biag")
            nc.vector.tensor_tensor(out=bias_g, in0=gnb_sb, in1=tmp_b,
                                    op=mybir.AluOpType.subtract)
            # xn (vector, 2x mode) + residual r = x + proj_b (scalar)
            for ct in range(CT):
                nc.vector.tensor_scalar(
                    out=xn_sb[:, ct, :], in0=x_sb[:, ct, :],
                    scalar1=sc_g[:, ct:ct + 1], scalar2=bias_g[:, ct:ct + 1],
                    op0=mybir.AluOpType.mult, op1=mybir.AluOpType.add)
            for ct in range(CT):
                nc.scalar.activation(
                    out=r_sb[:, ct, :], in_=x_sb[:, ct, :],
                    func=mybir.ActivationFunctionType.Identity,
                    bias=pb_sb[:, ct:ct + 1], scale=1.0)
            if debug:
                xn_f = dbgp.tile([128, N], F32, tag="dbgf")
                for ct in range(CT):
                    nc.vector.tensor_copy(xn_f, xn_sb[:, ct, :])
                    nc.sync.dma_start(out=dbg["d_xn"].rearrange(
                        "(t p) n -> p t n", p=128)[:, ct, :], in_=xn_f)

            # ---------------- emission helpers ----------------
            P = {}      # P[head][j] -> fp8 tile [128, 2, 1024] (j = st pair)
            hraw = {}   # hraw[head] -> bf16 tile [65, 1024]
            rsr = {}    # rsr[head] -> f32 [1, 1024] reciprocal rowsum
            rsb = {}    # broadcast [64, 1024]
            p1r = {}    # proj partial kt0-2 + r [128, 1024] bf16 per (ot)
            osb = {}

            def copy_on(e, out_, in_):
                if e == 's':
                    nc.scalar.activation(out=out_, in_=in_,
                                         func=mybir.ActivationFunctionType.Copy,
                                         bias=0.0, scale=1.0)
                elif e == 'g':
                    nc.gpsimd.tensor_copy(out_, in_)
                else:
                    nc.vector.tensor_copy(out_, in_)

            def exp_on(e, out_, in_):
                """out_bits = max(S + B8, 0) -> int8 == fp8e4m3 of 2^(S/A8*log2e)"""
                if e == 's':
                    nc.scalar.activation(out=out_, in_=in_,
                                         func=mybir.ActivationFunctionType.Relu,
                                         bias=b8_t, scale=1.0)
                else:
                    eng = nc.gpsimd if e == 'g' else nc.vector
                    eng.tensor_scalar(out=out_, in0=in_,
                                      scalar1=B8, scalar2=0.0,
                                      op0=mybir.AluOpType.add,
                                      op1=mybir.AluOpType.max)

            def qk_chain(hg, qk, sub, ch, eng):
                """one QK chain -> psum -> fp8 cast into q_sb/k_sb."""
                dst = q_sb if qk == 0 else k_sb
                blk = qk * 4 + hg * 2 + sub
                pt = ps_mm.tile([128, 512], F32, tag="mm")
                for kt in range(CT):
                    nc.tensor.matmul(
                        pt, qkvw_sb[:, kt, blk, :],
                        xn_sb[:, kt, ch * 512:(ch + 1) * 512],
                        start=(kt == 0), stop=(kt == CT - 1))
                copy_on(eng, dst[:, hg, sub, ch * 512:(ch + 1) * 512], pt)

            def vt_mm(st, eng):
                pt = ps_mm.tile([128, 512], F32, tag="mm")
                for kt in range(CT):
                    nc.tensor.matmul(
                        pt, xn_sb[:, kt, st * 128:(st + 1) * 128],
                        qkvw_sb[:, kt, 8:12, :].rearrange("p b o -> p (b o)"),
                        start=(kt == 0), stop=(kt == CT - 1))
                copy_on(eng, vt_sb[:, st, :, 0:64],
                        pt.rearrange("p (h c) -> p h c", h=NH))

            def s_exp(h, st, eng):
                """S DoubleRow matmuls for (head, st) + exp bit-trick."""
                hg, hb = h // 4, h % 4
                base = hb * 32
                spt = ps_s.tile([128, N], F32, tag="s")
                for ch in range(NCH):
                    nc.tensor.matmul(
                        spt[:, ch * 512:(ch + 1) * 512],
                        k_sb[base:base + 32, hg, :, st * 128:(st + 1) * 128],
                        q_sb[base:base + 32, hg, :, ch * 512:(ch + 1) * 512],
                        start=True, stop=True,
                        perf_mode=mybir.MatmulPerfMode.DoubleRow,
                        tile_position=(base, 0))
                j, parity = st // 2, st % 2
                if j not in P.setdefault(h, {}):
                    P[h][j] = pp.tile([128, 2, N], FP8, name=f"P{h}_{j}", tag="P")
                exp_on(eng, P[h][j][:, parity, :].bitcast(I8), spt)

            def dump_p0():
                for j in range(4):
                    for a in range(2):
                        pf2 = dbgp.tile([128, N], F32, name=f"pf{j}{a}", tag="dbgf")
                        nc.vector.tensor_copy(pf2, P[0][j][:, a, :])
                        nc.sync.dma_start(out=dbg["d_p"].rearrange(
                            "p (j a n) -> p j a n", j=4, a=2)[:, j, a, :], in_=pf2)

            def h_unit(h, engs):
                """H DoubleRow chains + rowsum recip + hraw casts + bcast + norm."""
                hraw[h] = hrawp.tile([64, N], BF16, name=f"hraw{h}", tag="hraw")
                rsf = rsp.tile([1, N], F32, name=f"rsf{h}", tag="rsf")
                rsr[h] = rsp.tile([1, N], F32, name=f"rsr{h}", tag="rsr")
                for ch in range(NCH):
                    hpt = ps_h.tile([80, 512], F32, tag="hps")
                    for j in range(4):
                        nc.tensor.matmul(
                            hpt, vt_sb[:, 2 * j:2 * j + 2, h, :],
                            P[h][j][:, :, ch * 512:(ch + 1) * 512],
                            start=(j == 0), stop=(j == 3),
                            perf_mode=mybir.MatmulPerfMode.DoubleRow)
                    # rowsum row -> SBUF f32 (recip from PSUM p64 is broken on HW)
                    copy_on(engs[1 - ch], rsf[:, ch * 512:(ch + 1) * 512],
                            hpt[64:65, :])
                    copy_on(engs[ch],
                            hraw[h][0:64, ch * 512:(ch + 1) * 512], hpt[0:64, :])
                nc.vector.reciprocal_approx_fast(out=rsr[h], in_=rsf)
                # broadcast 1/rowsum to 64 partitions (sbuf->sbuf DMA)
                rsb[h] = rsbp.tile([64, N], F32, name=f"rsb{h}", tag="rsb")
                nc.gpsimd.partition_broadcast(rsb[h], rsr[h], channels=64)
                # normalize into h_sb
                nc.vector.tensor_tensor(
                    out=h_sb[(h % 2) * 64:(h % 2) * 64 + 64, h // 2, :],
                    in0=hraw[h][0:64, :], in1=rsb[h],
                    op=mybir.AluOpType.mult)
                if debug and h == 0:
                    hf = dbgp.tile([64, N], F32, name="hf0", tag="dbgf")
                    nc.vector.tensor_copy(hf, hraw[h][0:64, :])
                    nc.sync.dma_start(out=dbg["d_hraw"][0:64, :], in_=hf)
                    rf = dbgp.tile([1, N], F32, name="rf0", tag="dbgf")
                    nc.vector.tensor_copy(rf, rsr[h])
                    nc.sync.dma_start(out=dbg["d_rs"], in_=rf)
                    rbf = dbgp.tile([64, N], F32, name="rbf0", tag="dbgf")
                    nc.vector.tensor_copy(rbf, rsb[h])
                    nc.sync.dma_start(out=dbg["d_rsb"], in_=rbf)

            def projA(ot, ch):
                """proj chain kt0..kt2 -> + r -> p1r bf16."""
                pt = ps_mm.tile([128, 512], F32, tag="mm")
                for kt in range(3):
                    nc.tensor.matmul(
                        pt, pw_sb[:, kt, ot * 128:(ot + 1) * 128],
                        h_sb[:, kt, ch * 512:(ch + 1) * 512],
                        start=(kt == 0), stop=(kt == 2))
                if ot not in p1r:
                    p1r[ot] = p1p.tile([128, N], BF16, name=f"p1r{ot}", tag="p1r")
                nc.vector.tensor_tensor(
                    out=p1r[ot][:, ch * 512:(ch + 1) * 512], in0=pt,
                    in1=r_sb[:, ot, ch * 512:(ch + 1) * 512],
                    op=mybir.AluOpType.add)

            def projB(ot, ch):
                """final kt3 matmul + add partial -> out tile."""
                pt = ps_mm.tile([128, 512], F32, tag="mm")
                nc.tensor.matmul(
                    pt, pw_sb[:, 3, ot * 128:(ot + 1) * 128],
                    h_sb[:, 3, ch * 512:(ch + 1) * 512],
                    start=True, stop=True)
                if ot not in osb:
                    osb[ot] = op_.tile([128, N], BF16, name=f"osb{ot}", tag="osb")
                nc.vector.tensor_tensor(
                    out=osb[ot][:, ch * 512:(ch + 1) * 512], in0=pt,
                    in1=p1r[ot][:, ch * 512:(ch + 1) * 512],
                    op=mybir.AluOpType.add)
                if ch == NCH - 1:
                    nc.sync.dma_start(out=out_t[:, ot, :], in_=osb[ot])

            # ---------------- schedule ----------------
            # Phase 1: QK half 0 (heads 0-3), casts alternate scalar/vector
            ce = ['s', 'v']
            i = 0
            for sub in range(2):
                for qk in range(2):
                    for ch in range(NCH):
                        qk_chain(0, qk, sub, ch, ce[i % 2]); i += 1

            # Phase 2: S(0), S(1) rounds + VT
            ee = ['v', 's']
            for st in range(ST):
                s_exp(0, st, ee[st % 2])
                s_exp(1, st, ee[(st + 1) % 2])
                vt_mm(st, 's' if st % 2 == 0 else 'v')

            # Phase 3: S(2), S(3) rounds + QK half 1 + H(0), H(1)
            qk1 = [(sub, qk, ch) for sub in range(2) for qk in range(2)
                   for ch in range(NCH)]
            for st in range(ST):
                s_exp(2, st, ee[st % 2])
                s_exp(3, st, ee[(st + 1) % 2])
                sub, qk, ch = qk1[st]
                qk_chain(1, qk, sub, ch, 'v' if st % 2 == 0 else 's')
                if st == 3:
                    if debug:
                        dump_p0()
                    h_unit(0, ('s', 'v'))
                elif st == 7:
                    h_unit(1, ('s', 'v'))

            # Phase 4: S(4), S(5) rounds + H(2), H(3)
            for st in range(ST):
                s_exp(4, st, ee[st % 2])
                s_exp(5, st, ee[(st + 1) % 2])
                if st == 3:
                    h_unit(2, ('s', 'v'))
                elif st == 7:
                    h_unit(3, ('s', 'v'))

            # Phase 5: S(6), S(7) rounds + H(4), H(5) + projA
            for st in range(ST):
                s_exp(6, st, ee[st % 2])
                s_exp(7, st, ee[(st + 1) % 2])
                if st == 1:
                    h_unit(4, ('s', 'v'))
                elif st == 3:
                    h_unit(5, ('s', 'v'))
                elif st >= 4:
                    projA(st - 4, 0)
                    projA(st - 4, 1)

            # Phase 6: H(6), H(7), projB
            h_unit(6, ('s', 'v'))
            h_unit(7, ('s', 'v'))
            for ot in range(CT):
                projB(ot, 0)
                projB(ot, 1)

            if debug:
                for ct in range(CT):
                    f = dbgp.tile([128, N], F32, tag="dbgh")
                    nc.vector.tensor_copy(f, h_sb[:, ct, :])
                    nc.sync.dma_start(out=dbg["d_h"].rearrange(
                        "(t p) n -> p t n", p=128)[:, ct, :], in_=f)
                for hg in range(2):
                    for sub in range(2):
                        qf = dbgp.tile([128, N], F32, name=f"qf{hg}{sub}", tag="dbgf")
                        nc.vector.tensor_copy(qf, q_sb[:, hg, sub, :])
                        nc.sync.dma_start(out=dbg["d_q"].rearrange(
                            "p (a n) -> p a n", a=4)[:, hg * 2 + sub, :], in_=qf)
                        kf = dbgp.tile([128, N], F32, name=f"kf{hg}{sub}", tag="dbgf")
                        nc.vector.tensor_copy(kf, k_sb[:, hg, sub, :])
                        nc.sync.dma_start(out=dbg["d_k"].rearrange(
                            "p (a n) -> p a n", a=4)[:, hg * 2 + sub, :], in_=kf)
                for st in range(ST):
                    vf = dbgp.tile([128, NH * 80], F32, name=f"vf{st}", tag="dbgf")
                    nc.vector.tensor_copy(
                        vf.rearrange("p (h c) -> p h c", h=NH), vt_sb[:, st, :, :])
                    nc.sync.dma_start(out=dbg["d_vt"].rearrange(
                        "p (s c) -> p s c", s=ST)[:, st, :], in_=vf)

    nc.finalize()
    return nc


def make_in_maps(x, gn_w, gn_b, qkv_w, proj_w, proj_b):
    x = np.asarray(x, dtype=np.float32).reshape(B, C, N)
    gn_w = np.asarray(gn_w, dtype=np.float32)
    gn_b = np.asarray(gn_b, dtype=np.float32)
    qkv_w = np.asarray(qkv_w, dtype=np.float32)
    proj_w = np.asarray(proj_w, dtype=np.float32)
    proj_b = np.asarray(proj_b, dtype=np.float32)

    scale = 1.0 / np.sqrt(np.sqrt(HD))
    rows = qkv_w.reshape(NH, 3, HD, C)
    qw = rows[:, 0].reshape(NH, HD, C) * (scale * A8)   # exp-scale folded
    kw = rows[:, 1].reshape(NH, HD, C) * scale
    vw = rows[:, 2].reshape(C, C)

    # q/k col blocks: blk = qk*4 + hg*2 + sub; within blk m in [0,128):
    # head = hg*4 + m//32, dim = sub*32 + m%32
    blocks = []
    for w in (qw, kw):
        for hg in range(2):
            for sub in range(2):
                blk = np.empty((128, C), np.float32)
                for m in range(128):
                    head = hg * 4 + m // 32
                    dim = sub * 32 + m % 32
                    blk[m] = w[head, dim]
                blocks.append(blk)
    blocks.append(vw)        # natural head-major, 512 cols = 4 blocks
    wall = np.concatenate(blocks, axis=0)         # (12*128, C)
    qkvw_t = np.ascontiguousarray(wall.T).astype(ml_dtypes.bfloat16)

    pw_t = np.ascontiguousarray(proj_w.T).astype(ml_dtypes.bfloat16)
    gnw_dev = np.ascontiguousarray(gn_w.reshape(CT, 128).T)
    gnb_dev = np.ascontiguousarray(gn_b.reshape(CT, 128).T)
    pb_dev = np.ascontiguousarray(proj_b.reshape(CT, 128).T)
    mask = np.zeros((128, 128), dtype=np.float32)
    for g in range(8):
        mask[g * GS:(g + 1) * GS, g * GS:(g + 1) * GS] = 1.0 / GS

    in_maps = []
    for b in range(B):
        xc = np.ascontiguousarray(x[b])
        in_maps.append({
            "x": xc.astype(ml_dtypes.bfloat16),
            "qkvw": qkvw_t, "pw": pw_t,
            "gnw": gnw_dev, "gnb": gnb_dev, "pb": pb_dev, "mask": mask,
        })
    return in_maps


def kernel(x, gn_w, gn_b, qkv_w, proj_w, proj_b, num_heads):
    assert int(num_heads) == NH
    _install_ntff_hook()
    in_maps = make_in_maps(x, gn_w, gn_b, qkv_w, proj_w, proj_b)
    if "nc" not in _CACHE:
        _CACHE["nc"] = build_nc(debug=DEBUG)
    r = run_bass_kernel_spmd(_CACHE["nc"], in_maps,
                             core_ids=list(range(NCORES)), trace=TRACE)
    _CACHE["last_result"] = r
    out = np.stack([np.asarray(r.results[b]["out"], dtype=np.float32)
                    for b in range(B)])
    return out.reshape(B, C, 32, 32)
